# revision 5
# baseline (speedup 1.0000x reference)
"""Trainium2 Bass kernel for NanochatAttention (sliding-window GQA attention).

Sharding: 8 cores = (batch b in {0,1}) x (kv-group g in {0..3}).
Each core handles one batch's full sequence for one KV head and its 4 Q heads:
  - q/k/v/gate projections (Megatron column-parallel slices of wq/wk/wv/wgate)
  - RoPE + QK RMS-norm, value-embedding gate
  - sliding-window causal attention (window 512) for 4 heads
  - row-parallel out-projection slice -> partial [T, E] output
Host sums the 4 partials per batch (row-parallel allreduce done on host at
unshard time).

Layout strategy (per core):
  xT   [E, T]  bf16  (host-transposed)  -- contraction dim E on partitions
  q/k/v projected in natural [t, d] tiles (PSUM), RoPE/RMS in [t, d] where the
  head dim is free and stats are per-partition scalars; q/k then transposed
  128x128 on the PE into [d, t] for the scores matmul.  Softmax runs without
  max-subtraction (QK-norm bounds |scores| <= sqrt(128)); masking via
  tensor_mask_reduce (masked -> -3.4e38 -> exp -> 0).  P is transposed via a
  regular matmul against diag(1/rowsum), which folds the softmax division in
  for free.  PV produces yT [d, t] directly, which is exactly the lhsT the
  out-projection needs.
"""

import numpy as np
import ml_dtypes

import concourse.bass as bass
import concourse.bacc as bacc
import concourse.tile as tile
from concourse import mybir
from concourse import bass_utils

BF = mybir.dt.bfloat16
F32 = mybir.dt.float32
AF = mybir.ActivationFunctionType
ALU = mybir.AluOpType

B = 2
T = 2048
E = 2048
D = 128          # head dim
HQ = 4           # q heads per core (one kv group)
NKV = 4
NT = T // 128    # 16 t-tiles
NE = E // 128    # 16 e-tiles
W = 512          # sliding window
NJB = W // 128   # 4 blocks of history + 1 current
EPS = float(np.finfo(np.float32).eps)


def _bcast_mid(ap, n):
    """Insert a step-0 dim after the partition dim: [p, w] -> [p, n, w]."""
    return bass.AP(tensor=ap.tensor, offset=ap.offset,
                   ap=[ap.ap[0], [0, n], *ap.ap[1:]])


def _body(tc, io, stage=99):
    nc = tc.nc
    xT, wq, wkvg, wo, ve2, cosd, sind, masks, ident, out = (
        io["xT"], io["wq"], io["wkvg"], io["wo"], io["ve2"], io["cos"],
        io["sin"], io["masks"], io["ident"], io["out"])

    with (
        tc.tile_pool(name="const", bufs=1) as cpool,
        tc.tile_pool(name="kv", bufs=1) as kvpool,
        tc.tile_pool(name="work", bufs=2) as work,
        tc.tile_pool(name="attn", bufs=3) as attn,
        tc.tile_pool(name="outp", bufs=2) as outp,
        tc.tile_pool(name="ps_proj", bufs=2, space="PSUM") as ps_proj,
        tc.tile_pool(name="ps_kvg", bufs=1, space="PSUM") as ps_kvg,
        tc.tile_pool(name="ps_spt", bufs=2, space="PSUM") as ps_spt,
        tc.tile_pool(name="ps_small", bufs=1, space="PSUM") as ps_small,
    ):
        # ---- resident loads -------------------------------------------------
        xT_sb = cpool.tile([128, NE, T], BF, tag="xT")
        for e in range(NE):
            nc.sync.dma_start(xT_sb[:, e, :], xT[e * 128:(e + 1) * 128, :])
        wq_sb = cpool.tile([128, NE, HQ * D], BF, tag="wq")
        nc.sync.dma_start(wq_sb, wq.rearrange("(e p) f -> p e f", p=128))
        wkvg_sb = cpool.tile([128, NE, 257], BF, tag="wkvg")
        nc.sync.dma_start(wkvg_sb, wkvg.rearrange("(e p) f -> p e f", p=128))
        wo_sb = cpool.tile([128, HQ, E], BF, tag="wo")
        nc.sync.dma_start(wo_sb, wo.rearrange("(h p) e -> p h e", p=128))
        ve_sb = cpool.tile([128, NT, D], BF, tag="ve")
        nc.sync.dma_start(ve_sb, ve2.rearrange("(t p) d -> p t d", p=128))
        cos_sb = cpool.tile([128, NT, 64], F32, tag="cos")
        nc.sync.dma_start(cos_sb, cosd.rearrange("(t p) h -> p t h", p=128))
        sin_sb = cpool.tile([128, NT, 64], F32, tag="sin")
        nc.sync.dma_start(sin_sb, sind.rearrange("(t p) h -> p t h", p=128))
        masks_sb = cpool.tile([128, 2, 128], F32, tag="masks")
        nc.sync.dma_start(masks_sb, masks.rearrange("m p j -> p m j"))
        ident_sb = cpool.tile([128, 128], BF, tag="ident")
        nc.sync.dma_start(ident_sb, ident)

        # kT (post rope+rms, [d, t]) and v ([t, d]) for the whole sequence
        kT_all = kvpool.tile([128, NT, 128], BF, tag="kT")
        v_all = kvpool.tile([128, NT, D], BF, tag="v")
        # per-(t-tile, head) exp scale = rsqrt(sum_d q_roped^2 + 128*eps)
        qsc_all = kvpool.tile([128, NT, HQ], F32, tag="qsc")

        for tt in range(NT):
            ts = slice(tt * 128, (tt + 1) * 128)
            if stage < 1:
                continue
            # ---- projections ------------------------------------------------
            psq = ps_proj.tile([128, HQ * D], F32, tag="psq")
            pskvg = ps_kvg.tile([128, 257], F32, tag="pskvg")
            for e in range(NE):
                nc.tensor.matmul(psq, xT_sb[:, e, ts], wq_sb[:, e, :],
                                 start=(e == 0), stop=(e == NE - 1))
            for e in range(NE):
                nc.tensor.matmul(pskvg, xT_sb[:, e, ts], wkvg_sb[:, e, :],
                                 start=(e == 0), stop=(e == NE - 1))

            if stage < 2:
                dbg = work.tile([128, 128], F32, tag="dbg")
                nc.vector.tensor_copy(dbg, psq[:, 0:128])
                nc.vector.tensor_copy(dbg, pskvg[:, 0:128])
                continue
            # ---- RoPE (q: [128, 4, 128] view; k: [128, 128]) ---------------
            q3 = psq.rearrange("p (h d) -> p h d", h=HQ)
            cosb = _bcast_mid(cos_sb[:, tt, :], HQ)
            sinb = _bcast_mid(sin_sb[:, tt, :], HQ)
            qro = work.tile([128, HQ, D], F32, tag="qro")
            ra = work.tile([128, HQ, 64], F32, tag="ra")
            rb = work.tile([128, HQ, 64], F32, tag="rb")
            rc = work.tile([128, HQ, 64], F32, tag="rc")
            rd = work.tile([128, HQ, 64], F32, tag="rd")
            nc.vector.tensor_mul(ra, q3[:, :, 0:64], cosb)
            nc.vector.tensor_mul(rb, q3[:, :, 64:128], sinb)
            nc.vector.tensor_add(qro[:, :, 0:64], ra, rb)
            nc.vector.tensor_mul(rc, q3[:, :, 64:128], cosb)
            nc.vector.tensor_mul(rd, q3[:, :, 0:64], sinb)
            nc.vector.tensor_sub(qro[:, :, 64:128], rc, rd)

            kro = work.tile([128, D], F32, tag="kro")
            ka = work.tile([128, 64], F32, tag="ka")
            kb = work.tile([128, 64], F32, tag="kb")
            kc = work.tile([128, 64], F32, tag="kc")
            kd = work.tile([128, 64], F32, tag="kd")
            nc.vector.tensor_mul(ka, pskvg[:, 0:64], cos_sb[:, tt, :])
            nc.vector.tensor_mul(kb, pskvg[:, 64:128], sin_sb[:, tt, :])
            nc.vector.tensor_add(kro[:, 0:64], ka, kb)
            nc.vector.tensor_mul(kc, pskvg[:, 64:128], cos_sb[:, tt, :])
            nc.vector.tensor_mul(kd, pskvg[:, 0:64], sin_sb[:, tt, :])
            nc.vector.tensor_sub(kro[:, 64:128], kc, kd)

            # ---- RMS stats --------------------------------------------------
            sq = work.tile([128, HQ * D], F32, tag="sq")
            qss = work.tile([128, HQ], F32, tag="qss")
            for h in range(HQ):
                nc.scalar.activation(sq[:, h * D:(h + 1) * D], qro[:, h, :],
                                     AF.Square, accum_out=qss[:, h:h + 1])
            # qsc = rsqrt(qss + 128*eps) = rsqrt(ms+eps)/sqrt(128)
            qtmp = work.tile([128, HQ], F32, tag="qtmp")
            nc.vector.tensor_scalar_add(qtmp, qss, float(128.0 * EPS))
            qrec = work.tile([128, HQ], F32, tag="qrec")
            nc.vector.reciprocal(qrec, qtmp)
            nc.scalar.activation(qsc_all[:, tt, :], qrec, AF.Sqrt)

            ksq = work.tile([128, D], F32, tag="ksq")
            kss = work.tile([128, 1], F32, tag="kss")
            nc.scalar.activation(ksq, kro, AF.Square, accum_out=kss)
            ktmp = work.tile([128, 1], F32, tag="ktmp")
            # kscale = rsqrt(kss/128 + eps) = sqrt(128) * rsqrt(kss + 128*eps)
            nc.vector.tensor_scalar(ktmp, kss, float(1.0 / 128.0), EPS,
                                    op0=ALU.mult, op1=ALU.add)
            krec = work.tile([128, 1], F32, tag="krec")
            nc.vector.reciprocal(krec, ktmp)
            ksc = work.tile([128, 1], F32, tag="ksc")
            nc.scalar.activation(ksc, krec, AF.Sqrt)
            krms = work.tile([128, D], BF, tag="krms")
            nc.vector.tensor_scalar_mul(krms, kro, ksc)

            # ---- v = vproj + sigmoid(gate_logit) * (2*ve) -------------------
            sig = work.tile([128, 1], F32, tag="sig")
            nc.scalar.activation(sig, pskvg[:, 256:257], AF.Sigmoid)
            nc.vector.scalar_tensor_tensor(
                v_all[:, tt, :], ve_sb[:, tt, :], sig, pskvg[:, 128:256],
                op0=ALU.mult, op1=ALU.add)

            if stage < 3:
                continue
            # ---- cast q to bf16, transpose q/k to [d, t] --------------------
            qbf = work.tile([128, HQ * D], BF, tag="qbf")
            nc.gpsimd.tensor_copy(qbf, qro.rearrange("p h d -> p (h d)"))
            qT = work.tile([128, HQ, 128], BF, tag="qT")
            for h in range(HQ):
                trp = ps_small.tile([128, 128], F32, tag="sm")
                nc.tensor.matmul(trp, qbf[:, h * D:(h + 1) * D], ident_sb,
                                 start=True, stop=True)
                nc.scalar.copy(qT[:, h, :], trp)
            trk = ps_small.tile([128, 128], F32, tag="sm")
            nc.tensor.matmul(trk, krms, ident_sb, start=True, stop=True)
            nc.scalar.copy(kT_all[:, tt, :], trk)

            # ---- attention for this q tile ----------------------------------
            if stage < 4:
                continue
            njb = min(tt, NJB) + 1
            jb0 = tt - (njb - 1)
            yT = outp.tile([128, HQ, 128], BF, tag="yT")
            for h in range(HQ):
                s_ps = ps_spt.tile([128, NJB + 1, 128], F32, tag="spt")
                nhist = njb - 1
                if nhist > 0:
                    nc.tensor.matmul(
                        s_ps[:, 0:nhist, :].rearrange("p a b -> p (a b)"),
                        qT[:, h, :],
                        kT_all[:, jb0:jb0 + nhist, :].rearrange(
                            "p a b -> p (a b)"),
                        start=True, stop=True)
                nc.tensor.matmul(s_ps[:, nhist, :], qT[:, h, :],
                                 kT_all[:, tt, :], start=True, stop=True)
                # masks: oldest block (window edge) keeps j_rel >= i+1;
                # newest block (causal) keeps j_rel <= i.
                if stage < 5:
                    continue
                # additive masks: [0]=window edge (-1e5 where j<=i),
                # [1]=causal (-1e5 where j>i)
                if tt >= NJB:
                    nc.vector.tensor_add(s_ps[:, 0, :], s_ps[:, 0, :],
                                         masks_sb[:, 0, :])
                nc.vector.tensor_add(s_ps[:, nhist, :], s_ps[:, nhist, :],
                                     masks_sb[:, 1, :])

                if stage < 6:
                    continue
                pexp = attn.tile([128, NJB + 1, 128], BF, tag="pexp")
                ssum = attn.tile([128, 1], F32, tag="ssum")
                nc.scalar.activation(
                    pexp[:, 0:njb, :].rearrange("p a b -> p (a b)"),
                    s_ps[:, 0:njb, :].rearrange("p a b -> p (a b)"),
                    AF.Exp, scale=qsc_all[:, tt, h:h + 1], accum_out=ssum)
                rsum = attn.tile([128, 1], F32, tag="rsum")
                nc.vector.reciprocal(rsum, ssum)
                diag = attn.tile([128, 128], BF, tag="diag")
                nc.vector.tensor_scalar_mul(diag, ident_sb, rsum)

                if stage < 7:
                    continue
                # PT[j_rel, i] = P[i, j_rel] / rowsum(i), per j-block
                pt_ps = ps_spt.tile([128, NJB + 1, 128], F32, tag="spt")
                for jb in range(njb):
                    nc.tensor.matmul(pt_ps[:, jb, :], pexp[:, jb, :], diag,
                                     start=True, stop=True)
                pt_sb = attn.tile([128, NJB + 1, 128], BF, tag="ptsb")
                if h % 2 == 0:
                    nc.scalar.copy(
                        pt_sb[:, 0:njb, :].rearrange("p a b -> p (a b)"),
                        pt_ps[:, 0:njb, :].rearrange("p a b -> p (a b)"))
                else:
                    nc.vector.tensor_copy(
                        pt_sb[:, 0:njb, :].rearrange("p a b -> p (a b)"),
                        pt_ps[:, 0:njb, :].rearrange("p a b -> p (a b)"))

                if stage < 8:
                    continue
                yps = ps_small.tile([128, 128], F32, tag="sm")
                for jb in range(njb):
                    nc.tensor.matmul(yps, v_all[:, jb0 + jb, :],
                                     pt_sb[:, jb, :],
                                     start=(jb == 0), stop=(jb == njb - 1))
                if h % 2 == 0:
                    nc.vector.tensor_copy(yT[:, h, :], yps)
                else:
                    nc.scalar.copy(yT[:, h, :], yps)

            if stage < 9:
                continue
            # ---- out-projection: out[t, :] = sum_h yT_h.T @ wo_h ------------
            osb = outp.tile([128, E], F32, tag="osb")
            for ec in range(4):
                ops = ps_proj.tile([128, 512], F32, tag="psq")
                for h in range(HQ):
                    nc.tensor.matmul(ops, yT[:, h, :],
                                     wo_sb[:, h, ec * 512:(ec + 1) * 512],
                                     start=(h == 0), stop=(h == HQ - 1))
                if ec % 2 == 0:
                    nc.vector.tensor_copy(osb[:, ec * 512:(ec + 1) * 512], ops)
                else:
                    nc.scalar.copy(osb[:, ec * 512:(ec + 1) * 512], ops)
            nc.sync.dma_start(out[ts, :], osb)


def build_nc(stage=99):
    nc = bacc.Bacc("TRN2", target_bir_lowering=False, debug=False,
                   num_devices=8)
    io = {
        "xT": nc.dram_tensor("xT", [E, T], BF, kind="ExternalInput").ap(),
        "wq": nc.dram_tensor("wq", [E, HQ * D], BF, kind="ExternalInput").ap(),
        "wkvg": nc.dram_tensor("wkvg", [E, 257], BF, kind="ExternalInput").ap(),
        "wo": nc.dram_tensor("wo", [HQ * D, E], BF, kind="ExternalInput").ap(),
        "ve2": nc.dram_tensor("ve2", [T, D], BF, kind="ExternalInput").ap(),
        "cos": nc.dram_tensor("cos", [T, 64], F32, kind="ExternalInput").ap(),
        "sin": nc.dram_tensor("sin", [T, 64], F32, kind="ExternalInput").ap(),
        "masks": nc.dram_tensor("masks", [2, 128, 128], F32,
                                kind="ExternalInput").ap(),
        "ident": nc.dram_tensor("ident", [128, 128], BF,
                                kind="ExternalInput").ap(),
        "out": nc.dram_tensor("out", [T, E], F32, kind="ExternalOutput").ap(),
    }
    with tile.TileContext(nc) as tc:
        _body(tc, io, stage=stage)
    nc.compile()
    return nc


_NC = None


def _get_nc():
    global _NC
    if _NC is None:
        _NC = build_nc()
    return _NC


def _prep_in_maps(x, ve, cos, sin, wq, wk, wv, wo, wgate):
    x = np.asarray(x, dtype=np.float32)
    ve = np.asarray(ve, dtype=np.float32)
    cos2 = np.ascontiguousarray(np.asarray(cos, np.float32).reshape(T, 64))
    sin2 = np.ascontiguousarray(np.asarray(sin, np.float32).reshape(T, 64))
    ii = np.arange(128)
    masks = np.zeros((2, 128, 128), np.float32)
    masks[0][ii[:, None] >= ii[None, :]] = -1e5   # window edge: kill j <= i
    masks[1][ii[:, None] < ii[None, :]] = -1e5    # causal: kill j > i
    masks = np.ascontiguousarray(masks)
    ident = np.eye(128, dtype=ml_dtypes.bfloat16)

    xT_b = [np.ascontiguousarray(x[b].T).astype(ml_dtypes.bfloat16)
            for b in range(B)]
    in_maps = []
    for c in range(8):
        b, g = divmod(c, NKV)
        wq_c = np.ascontiguousarray(
            wq[g * 512:(g + 1) * 512, :].T).astype(ml_dtypes.bfloat16)
        wk_c = wk[g * 128:(g + 1) * 128, :].T
        wv_c = wv[g * 128:(g + 1) * 128, :].T
        gcol = np.zeros((E, 1), np.float32)
        gcol[:32, 0] = wgate[g]
        wkvg_c = np.ascontiguousarray(
            np.concatenate([wk_c, wv_c, gcol], axis=1)).astype(
                ml_dtypes.bfloat16)
        wo_c = np.ascontiguousarray(
            wo[:, g * 512:(g + 1) * 512].T).astype(ml_dtypes.bfloat16)
        ve2_c = np.ascontiguousarray(
            2.0 * ve[b, :, g * 128:(g + 1) * 128]).astype(ml_dtypes.bfloat16)
        in_maps.append({
            "xT": xT_b[b], "wq": wq_c, "wkvg": wkvg_c, "wo": wo_c,
            "ve2": ve2_c, "cos": cos2, "sin": sin2, "masks": masks,
            "ident": ident,
        })
    return in_maps


def kernel(x, ve, cos, sin, wq, wk, wv, wo, wgate, window_size=512,
           _trace=False):
    assert int(window_size) == W, f"kernel hardcodes window {W}"
    wq = np.asarray(wq, np.float32)
    wk = np.asarray(wk, np.float32)
    wv = np.asarray(wv, np.float32)
    wo = np.asarray(wo, np.float32)
    wgate = np.asarray(wgate, np.float32)
    in_maps = _prep_in_maps(x, ve, cos, sin, wq, wk, wv, wo, wgate)
    nc = _get_nc()
    res = bass_utils.run_bass_kernel_spmd(
        nc, in_maps, core_ids=list(range(8)), trace=_trace)
    out = np.empty((B, T, E), np.float32)
    for b in range(B):
        acc = res.results[b * NKV]["out"].astype(np.float32).copy()
        for g in range(1, NKV):
            acc += res.results[b * NKV + g]["out"]
        out[b] = acc
    if _trace:
        kernel.last_results = res
    return out


# revision 6
# speedup vs baseline: 2.0923x; 2.0923x over previous
"""Trainium2 Bass kernel for NanochatAttention (sliding-window GQA attention).

Sharding: 8 cores = (batch b in {0,1}) x (kv-group g in {0..3}).
Each core handles one batch's full sequence for one KV head and its 4 Q heads:
projections, RoPE + QK RMS-norm, value-embedding gate, 512-window causal
attention, and the row-parallel out-projection slice -> partial [T, E] output.
Host sums the 4 partials per batch at unshard time.

Three dense phases (keeps the PE clock warm and ACT tables stable):
  A: projections (PSUM) -> RoPE/RMS stats -> q/k transposes to [d, t]
  B: attention per (t-tile, head): scores + additive-mask matmuls -> exp
     (scale = per-row qscale, sums via accum) -> P transposed AND divided by
     rowsum in one matmul against diag(1/s) -> PV -> yT [d, t]
  C: out-projection (yT is exactly the lhsT it needs) -> DMA out
"""

import numpy as np
import ml_dtypes

import concourse.bass as bass
import concourse.bacc as bacc
import concourse.tile as tile
from concourse import mybir
from concourse import bass_utils

BF = mybir.dt.bfloat16
F32 = mybir.dt.float32
AF = mybir.ActivationFunctionType
ALU = mybir.AluOpType

B = 2
T = 2048
E = 2048
D = 128          # head dim
HQ = 4           # q heads per core (one kv group)
NKV = 4
NT = T // 128    # 16 t-tiles
NE = E // 128    # 16 e-tiles
W = 512          # sliding window
NJB = W // 128   # history blocks
EPS = float(np.finfo(np.float32).eps)
SQRT_D = float(np.sqrt(128.0))


def _bcast_mid(ap, n):
    """Insert a step-0 dim after the partition dim: [p, w] -> [p, n, w]."""
    return bass.AP(tensor=ap.tensor, offset=ap.offset,
                   ap=[ap.ap[0], [0, n], *ap.ap[1:]])


def _body(tc, io):
    nc = tc.nc
    xT, wq, wkvg, wo, ve2, cosd, sind, masks, ident, out = (
        io["xT"], io["wq"], io["wkvg"], io["wo"], io["ve2"], io["cos"],
        io["sin"], io["masks"], io["ident"], io["out"])

    with (
        tc.tile_pool(name="const", bufs=1) as cpool,
        tc.tile_pool(name="state", bufs=1) as state,
    ):
        # ---- resident constants ----------------------------------------
        wq_sb = cpool.tile([128, NE, HQ * D], BF, tag="wq")
        nc.sync.dma_start(wq_sb, wq.rearrange("(e p) f -> p e f", p=128))
        wkvg_sb = cpool.tile([128, NE, 257], BF, tag="wkvg")
        nc.sync.dma_start(wkvg_sb, wkvg.rearrange("(e p) f -> p e f", p=128))
        wo_sb = cpool.tile([128, HQ, E], BF, tag="wo")
        nc.sync.dma_start(wo_sb, wo.rearrange("(h p) e -> p h e", p=128))
        ve_sb = cpool.tile([128, NT, D], BF, tag="ve")
        nc.sync.dma_start(ve_sb, ve2.rearrange("(t p) d -> p t d", p=128))
        cos_sb = cpool.tile([128, NT, 64], F32, tag="cos")
        nc.sync.dma_start(cos_sb, cosd.rearrange("(t p) h -> p t h", p=128))
        sin_sb = cpool.tile([128, NT, 64], F32, tag="sin")
        nc.sync.dma_start(sin_sb, sind.rearrange("(t p) h -> p t h", p=128))
        masks_sb = cpool.tile([128, 2, 128], BF, tag="masks")
        nc.sync.dma_start(masks_sb, masks.rearrange("m p j -> p m j"))
        ident_sb = cpool.tile([128, 128], BF, tag="ident")
        nc.sync.dma_start(ident_sb, ident)

        # ---- whole-sequence state --------------------------------------
        kT_all = state.tile([128, NT, 128], BF, tag="kT")
        v_all = state.tile([128, NT, D], BF, tag="v")
        qT_all = state.tile([128, NT, HQ, 128], BF, tag="qT")
        yT_all = state.tile([128, NT, HQ, 128], BF, tag="yT")
        sc_all = state.tile([128, NT, HQ + 1], F32, tag="sc")
        vst = state.tile([128, NT, D], BF, tag="vst")
        glog = state.tile([128, NT], F32, tag="glog")

        # ================= PHASE A: proj + rope + rms + transposes ======
        with (
            tc.tile_pool(name="xp", bufs=1) as xp,
            tc.tile_pool(name="workA", bufs=2) as work,
            tc.tile_pool(name="psA_q", bufs=2, space="PSUM") as psA_q,
            tc.tile_pool(name="psA_kvg", bufs=2, space="PSUM") as psA_kvg,
            tc.tile_pool(name="psA_tr", bufs=2, space="PSUM") as psA_tr,
        ):
            xT_sb = xp.tile([128, NE, T], BF, tag="xT")
            for e in range(NE):
                nc.sync.dma_start(xT_sb[:, e, :], xT[e * 128:(e + 1) * 128, :])

            for tt in range(NT):
                ts = slice(tt * 128, (tt + 1) * 128)
                psq = psA_q.tile([128, HQ * D], F32, tag="psq")
                pskvg = psA_kvg.tile([128, 257], F32, tag="pskvg")
                for e in range(NE):
                    nc.tensor.matmul(psq, xT_sb[:, e, ts], wq_sb[:, e, :],
                                     start=(e == 0), stop=(e == NE - 1))
                for e in range(NE):
                    nc.tensor.matmul(pskvg, xT_sb[:, e, ts], wkvg_sb[:, e, :],
                                     start=(e == 0), stop=(e == NE - 1))

                # RoPE q ([128, 4, 128] view) and k ([128, 128])
                q3 = psq.rearrange("p (h d) -> p h d", h=HQ)
                cosb = _bcast_mid(cos_sb[:, tt, :], HQ)
                sinb = _bcast_mid(sin_sb[:, tt, :], HQ)
                qro = work.tile([128, HQ, D], F32, tag="qro")
                ra = work.tile([128, HQ, 64], F32, tag="ra")
                rb = work.tile([128, HQ, 64], F32, tag="rb")
                rc = work.tile([128, HQ, 64], F32, tag="rc")
                rd = work.tile([128, HQ, 64], F32, tag="rd")
                nc.vector.tensor_mul(ra, q3[:, :, 0:64], cosb)
                nc.vector.tensor_mul(rb, q3[:, :, 64:128], sinb)
                nc.vector.tensor_add(qro[:, :, 0:64], ra, rb)
                nc.vector.tensor_mul(rc, q3[:, :, 64:128], cosb)
                nc.vector.tensor_mul(rd, q3[:, :, 0:64], sinb)
                nc.vector.tensor_sub(qro[:, :, 64:128], rc, rd)

                kro = work.tile([128, D], F32, tag="kro")
                ka = work.tile([128, 64], F32, tag="ka")
                kb = work.tile([128, 64], F32, tag="kb")
                kc = work.tile([128, 64], F32, tag="kc")
                kd = work.tile([128, 64], F32, tag="kd")
                nc.vector.tensor_mul(ka, pskvg[:, 0:64], cos_sb[:, tt, :])
                nc.vector.tensor_mul(kb, pskvg[:, 64:128], sin_sb[:, tt, :])
                nc.vector.tensor_add(kro[:, 0:64], ka, kb)
                nc.vector.tensor_mul(kc, pskvg[:, 64:128], cos_sb[:, tt, :])
                nc.vector.tensor_mul(kd, pskvg[:, 0:64], sin_sb[:, tt, :])
                nc.vector.tensor_sub(kro[:, 64:128], kc, kd)

                # RMS stats: sum of squares for 4 q heads + k (ACT, one table)
                sq = work.tile([128, (HQ + 1) * D], F32, tag="sq")
                qkss = work.tile([128, HQ + 1], F32, tag="qkss")
                for h in range(HQ):
                    nc.scalar.activation(sq[:, h * D:(h + 1) * D],
                                         qro[:, h, :], AF.Square,
                                         accum_out=qkss[:, h:h + 1])
                nc.scalar.activation(sq[:, HQ * D:], kro, AF.Square,
                                     accum_out=qkss[:, HQ:HQ + 1])
                # rsqrt(ss + 128*eps): q -> exp scale (1/sqrt(128) folded in),
                # k -> kscale/sqrt(128)
                tmp5 = work.tile([128, HQ + 1], F32, tag="tmp5")
                nc.vector.tensor_scalar_add(tmp5, qkss, float(128.0 * EPS))
                rec5 = work.tile([128, HQ + 1], F32, tag="rec5")
                nc.vector.reciprocal(rec5, tmp5)
                nc.scalar.activation(sc_all[:, tt, :], rec5, AF.Sqrt)

                # krms = kro * kscale = kro * sc_k * sqrt(128)  (bf16)
                krms = work.tile([128, D], BF, tag="krms")
                nc.vector.tensor_scalar(krms, kro, sc_all[:, tt, HQ:HQ + 1],
                                        SQRT_D, op0=ALU.mult, op1=ALU.mult)
                # stage v-proj + gate logit for the phase boundary
                nc.vector.tensor_copy(vst[:, tt, :], pskvg[:, 128:256])
                nc.vector.tensor_copy(glog[:, tt:tt + 1], pskvg[:, 256:257])

                # cast q to bf16 (gpsimd), transpose q/k to [d, t] on the PE
                qbf = work.tile([128, HQ * D], BF, tag="qbf")
                nc.gpsimd.tensor_copy(qbf, qro.rearrange("p h d -> p (h d)"))
                trp = psA_tr.tile([128, HQ + 1, 128], F32, tag="trp")
                for h in range(HQ):
                    nc.tensor.matmul(trp[:, h, :], qbf[:, h * D:(h + 1) * D],
                                     ident_sb, start=True, stop=True)
                nc.tensor.matmul(trp[:, HQ, :], krms, ident_sb,
                                 start=True, stop=True)
                nc.vector.tensor_copy(
                    qT_all[:, tt, :, :].rearrange("p a b -> p (a b)"),
                    trp[:, 0:HQ, :].rearrange("p a b -> p (a b)"))
                nc.vector.tensor_copy(kT_all[:, tt, :], trp[:, HQ, :])

        # ---- A->B boundary: batched sigmoid + v assembly ----------------
        with tc.tile_pool(name="bnd", bufs=1) as bnd:
            sig_all = bnd.tile([128, NT], F32, tag="sig")
            nc.scalar.activation(sig_all, glog, AF.Sigmoid)
            for tt in range(NT):
                nc.vector.scalar_tensor_tensor(
                    v_all[:, tt, :], ve_sb[:, tt, :], sig_all[:, tt:tt + 1],
                    vst[:, tt, :], op0=ALU.mult, op1=ALU.add)

            # ============= PHASE B: attention ============================
            with (
                tc.tile_pool(name="attn", bufs=4) as attn,
                tc.tile_pool(name="psB", bufs=4, space="PSUM") as psB,
            ):
                for tt in range(NT):
                    njb = min(tt, NJB) + 1
                    jb0 = tt - (njb - 1)
                    nhist = njb - 1
                    for h in range(HQ):
                        s_ps = psB.tile([128, NJB + 1, 128], F32, tag="spt")
                        qT_h = qT_all[:, tt, h, :]
                        if tt >= NJB:
                            # oldest block + window-edge mask, joint group
                            nc.tensor.matmul(s_ps[:, 0, :], qT_h,
                                             kT_all[:, jb0, :],
                                             start=True, stop=False)
                            nc.tensor.matmul(s_ps[:, 0, :], ident_sb,
                                             masks_sb[:, 0, :],
                                             start=False, stop=True)
                            if nhist > 1:
                                nc.tensor.matmul(
                                    s_ps[:, 1:nhist, :].rearrange(
                                        "p a b -> p (a b)"),
                                    qT_h,
                                    kT_all[:, jb0 + 1:tt, :].rearrange(
                                        "p a b -> p (a b)"),
                                    start=True, stop=True)
                        elif nhist > 0:
                            nc.tensor.matmul(
                                s_ps[:, 0:nhist, :].rearrange(
                                    "p a b -> p (a b)"),
                                qT_h,
                                kT_all[:, jb0:tt, :].rearrange(
                                    "p a b -> p (a b)"),
                                start=True, stop=True)
                        # current block + causal mask
                        nc.tensor.matmul(s_ps[:, nhist, :], qT_h,
                                         kT_all[:, tt, :],
                                         start=True, stop=False)
                        nc.tensor.matmul(s_ps[:, nhist, :], ident_sb,
                                         masks_sb[:, 1, :],
                                         start=False, stop=True)

                        pexp = attn.tile([128, NJB + 1, 128], BF, tag="pexp")
                        ssum = attn.tile([128, 1], F32, tag="ssum")
                        nc.scalar.activation(
                            pexp[:, 0:njb, :].rearrange("p a b -> p (a b)"),
                            s_ps[:, 0:njb, :].rearrange("p a b -> p (a b)"),
                            AF.Exp, scale=sc_all[:, tt, h:h + 1],
                            accum_out=ssum)
                        rsum = attn.tile([128, 1], F32, tag="rsum")
                        nc.vector.reciprocal(rsum, ssum)
                        diag = attn.tile([128, 128], BF, tag="diag")
                        nc.vector.tensor_scalar_mul(diag, ident_sb, rsum)

                        # PT[j, i] = P[i, j]/s(i): overwrite S slot per block
                        for jb in range(njb):
                            nc.tensor.matmul(s_ps[:, jb, :], pexp[:, jb, :],
                                             diag, start=True, stop=True)
                        pt_sb = attn.tile([128, NJB + 1, 128], BF, tag="ptsb")
                        nc.vector.tensor_copy(
                            pt_sb[:, 0:njb, :].rearrange("p a b -> p (a b)"),
                            s_ps[:, 0:njb, :].rearrange("p a b -> p (a b)"))
                        # PV: yT[d, i] accumulated over j-blocks in slot bank0
                        for jb in range(njb):
                            nc.tensor.matmul(s_ps[:, 0, :],
                                             v_all[:, jb0 + jb, :],
                                             pt_sb[:, jb, :],
                                             start=(jb == 0),
                                             stop=(jb == njb - 1))
                        nc.vector.tensor_copy(yT_all[:, tt, h, :],
                                              s_ps[:, 0, :])

            # ============= PHASE C: out-projection =======================
            with (
                tc.tile_pool(name="outc", bufs=2) as outc,
                tc.tile_pool(name="psC", bufs=6, space="PSUM") as psC,
            ):
                for tt in range(NT):
                    ts = slice(tt * 128, (tt + 1) * 128)
                    osb = outc.tile([128, E], F32, tag="osb")
                    for ec in range(4):
                        ops = psC.tile([128, 512], F32, tag="ops")
                        for h in range(HQ):
                            nc.tensor.matmul(
                                ops, yT_all[:, tt, h, :],
                                wo_sb[:, h, ec * 512:(ec + 1) * 512],
                                start=(h == 0), stop=(h == HQ - 1))
                        nc.scalar.copy(osb[:, ec * 512:(ec + 1) * 512], ops)
                    nc.sync.dma_start(out[ts, :], osb)


def build_nc(stage=99):
    nc = bacc.Bacc("TRN2", target_bir_lowering=False, debug=False,
                   num_devices=8)
    io = {
        "xT": nc.dram_tensor("xT", [E, T], BF, kind="ExternalInput").ap(),
        "wq": nc.dram_tensor("wq", [E, HQ * D], BF, kind="ExternalInput").ap(),
        "wkvg": nc.dram_tensor("wkvg", [E, 257], BF, kind="ExternalInput").ap(),
        "wo": nc.dram_tensor("wo", [HQ * D, E], BF, kind="ExternalInput").ap(),
        "ve2": nc.dram_tensor("ve2", [T, D], BF, kind="ExternalInput").ap(),
        "cos": nc.dram_tensor("cos", [T, 64], F32, kind="ExternalInput").ap(),
        "sin": nc.dram_tensor("sin", [T, 64], F32, kind="ExternalInput").ap(),
        "masks": nc.dram_tensor("masks", [2, 128, 128], BF,
                                kind="ExternalInput").ap(),
        "ident": nc.dram_tensor("ident", [128, 128], BF,
                                kind="ExternalInput").ap(),
        "out": nc.dram_tensor("out", [T, E], F32, kind="ExternalOutput").ap(),
    }
    with tile.TileContext(nc) as tc:
        _body(tc, io)
    nc.compile()
    return nc


_NC = None


def _get_nc():
    global _NC
    if _NC is None:
        _NC = build_nc()
    return _NC


def _prep_in_maps(x, ve, cos, sin, wq, wk, wv, wo, wgate):
    x = np.asarray(x, dtype=np.float32)
    ve = np.asarray(ve, dtype=np.float32)
    cos2 = np.ascontiguousarray(np.asarray(cos, np.float32).reshape(T, 64))
    sin2 = np.ascontiguousarray(np.asarray(sin, np.float32).reshape(T, 64))
    ii = np.arange(128)
    masks = np.zeros((2, 128, 128), np.float32)
    masks[0][ii[:, None] >= ii[None, :]] = -30000.0  # window edge: kill j <= i
    masks[1][ii[:, None] < ii[None, :]] = -30000.0   # causal: kill j > i
    masks = np.ascontiguousarray(masks).astype(ml_dtypes.bfloat16)
    ident = np.eye(128, dtype=ml_dtypes.bfloat16)

    xT_b = [np.ascontiguousarray(x[b].T).astype(ml_dtypes.bfloat16)
            for b in range(B)]
    in_maps = []
    for c in range(8):
        b, g = divmod(c, NKV)
        wq_c = np.ascontiguousarray(
            wq[g * 512:(g + 1) * 512, :].T).astype(ml_dtypes.bfloat16)
        wk_c = wk[g * 128:(g + 1) * 128, :].T
        wv_c = wv[g * 128:(g + 1) * 128, :].T
        gcol = np.zeros((E, 1), np.float32)
        gcol[:32, 0] = wgate[g]
        wkvg_c = np.ascontiguousarray(
            np.concatenate([wk_c, wv_c, gcol], axis=1)).astype(
                ml_dtypes.bfloat16)
        wo_c = np.ascontiguousarray(
            wo[:, g * 512:(g + 1) * 512].T).astype(ml_dtypes.bfloat16)
        ve2_c = np.ascontiguousarray(
            2.0 * ve[b, :, g * 128:(g + 1) * 128]).astype(ml_dtypes.bfloat16)
        in_maps.append({
            "xT": xT_b[b], "wq": wq_c, "wkvg": wkvg_c, "wo": wo_c,
            "ve2": ve2_c, "cos": cos2, "sin": sin2, "masks": masks,
            "ident": ident,
        })
    return in_maps


def kernel(x, ve, cos, sin, wq, wk, wv, wo, wgate, window_size=512,
           _trace=False):
    assert int(window_size) == W, f"kernel hardcodes window {W}"
    wq = np.asarray(wq, np.float32)
    wk = np.asarray(wk, np.float32)
    wv = np.asarray(wv, np.float32)
    wo = np.asarray(wo, np.float32)
    wgate = np.asarray(wgate, np.float32)
    in_maps = _prep_in_maps(x, ve, cos, sin, wq, wk, wv, wo, wgate)
    nc = _get_nc()
    res = bass_utils.run_bass_kernel_spmd(
        nc, in_maps, core_ids=list(range(8)), trace=_trace)
    out = np.empty((B, T, E), np.float32)
    for b in range(B):
        acc = res.results[b * NKV]["out"].astype(np.float32).copy()
        for g in range(1, NKV):
            acc += res.results[b * NKV + g]["out"]
        out[b] = acc
    if _trace:
        kernel.last_results = res
    return out
